# revision 5
# baseline (speedup 1.0000x reference)
"""Trainium2 Bass kernel for the reference MultiHeadAttention module.

Problem: B=32, T=512, D=1024, H=16, HD=64 (see reference semantics below).

Reference computation (note the unusual orientation: keys index rows,
queries index softmax axis, no 1/sqrt(d) scale):
    h  = x @ Wi + bi
    k/q/v = per-head h @ W{k,q,v}[h] + b (head-stacked weights)
    wei[b,h,t,s] = k[b,h,t,:] . q[b,h,s,:]      (t = key idx, s = query idx)
    wei masked to s <= t, softmax over s
    out = (wei @ v) concat-heads @ Wo + bo

Sharding: data-parallel over batch. Each of the 8 cores processes 4
batches (2048 tokens) with replicated weights; no collectives. Host
re-assembles the full [32,512,1024] output.

Device dataflow per core (all matmuls in float32r: full-rate PE with
~13-bit mantissa operands, fp32 PSUM accumulation):
  Phase A (per 512-token chunk, streaming):
    h0^T = Wi^T-chain from host-transposed x^T (feature-major),
    q^T / k^T (feature-major) and v (token-major) from h0^T,
    spilled to DRAM scratch (SBUF can't hold Wi+Wqkv+activations).
  Phase B (per batch of 512 tokens):
    S^T[s,t] per head via q^T/k^T slices (base-partition 0/64),
    causal mask = additive -60000 on the diagonal 128x128 block plus
    column-range restriction (s-tile i only computes t >= 128*i),
    P^T = exp(S^T) on ACT (no max-subtraction: |logits| <= ~55 is safe
    in fp32 since the reference softmax is unscaled and unsubtracted
    errors cancel exactly in the ratio),
    O^T = [V | 1]^T @ P^T per head (ones column yields the softmax
    denominator as row 64), reciprocal + partition_broadcast + multiply
    to normalize, then out = O^T-chain @ Wo token-major, DMA'd out.

All biases are handled host-side or folded:
  bi,bk,bq fold into per-feature adds on q^T/k^T (applied on-device only
  if nonzero; they are zero in setup_inputs), bv+bi fold into a constant
  row added on host after out_proj, bo added on host.
"""

import sys

sys.path.insert(0, "/opt/trn_rl_repo")

import numpy as np

import concourse.bacc as bacc
import concourse.mybir as mybir
from concourse import bass_utils
from concourse.tile import TileContext

F32 = mybir.dt.float32
F32R = mybir.dt.float32r
AF = mybir.ActivationFunctionType

B, T, D, H, HD = 32, 512, 1024, 16, 64
NCORES = 8
BN = B // NCORES          # batches per core = 4
TOK = BN * T              # tokens per core = 2048
NKT = D // 128            # 8 contraction tiles
NMC = TOK // 512          # 4 token chunks (phase A)
MASK_NEG = -60000.0       # exp(-60000 + |logit|) == 0 in fp32

_CACHE = {}


def _build(with_qk_bias: bool):
    nc = bacc.Bacc("TRN2", target_bir_lowering=False, debug=False,
                   num_devices=NCORES)

    xT = nc.dram_tensor("xT", [D, TOK], F32, kind="ExternalInput")
    wi = nc.dram_tensor("wi", [D, D], F32, kind="ExternalInput")
    wq = nc.dram_tensor("wq", [D, D], F32, kind="ExternalInput")
    wk = nc.dram_tensor("wk", [D, D], F32, kind="ExternalInput")
    wv = nc.dram_tensor("wv", [D, D], F32, kind="ExternalInput")
    wo = nc.dram_tensor("wo", [D, D], F32, kind="ExternalInput")
    tri = nc.dram_tensor("tri", [128, 128], F32, kind="ExternalInput")
    onesc = nc.dram_tensor("onesc", [128, H], F32, kind="ExternalInput")
    if with_qk_bias:
        bq2 = nc.dram_tensor("bq2", [128, NKT], F32, kind="ExternalInput")
        bk2 = nc.dram_tensor("bk2", [128, NKT], F32, kind="ExternalInput")
    out = nc.dram_tensor("out", [TOK, D], F32, kind="ExternalOutput")

    # DRAM scratch spills (feature-major q/k, token-major v)
    qT_d = nc.dram_tensor("qT_d", [D, TOK], F32, kind="Internal")
    kT_d = nc.dram_tensor("kT_d", [D, TOK], F32, kind="Internal")
    v_d = nc.dram_tensor("v_d", [TOK, D], F32, kind="Internal")

    with TileContext(nc) as tc:
        with tc.tile_pool(name="const", bufs=1) as cpool:
            tri_sb = cpool.tile([128, 128], F32, tag="tri")
            nc.sync.dma_start(tri_sb[:], tri[:])
            if with_qk_bias:
                bq_sb = cpool.tile([128, NKT], F32, tag="bq")
                bk_sb = cpool.tile([128, NKT], F32, tag="bk")
                nc.sync.dma_start(bq_sb[:], bq2[:])
                nc.sync.dma_start(bk_sb[:], bk2[:])

            # ---------------- Phase A: in_proj + QKV, spill to DRAM ----
            with tc.tile_pool(name="wA", bufs=1) as wpool, \
                 tc.tile_pool(name="actA", bufs=1) as apool, \
                 tc.tile_pool(name="psA", bufs=7, space="PSUM") as pspool:
                wi_sb = [wpool.tile([128, D], F32R, tag=f"wi{k}", name=f"wi{k}") for k in range(NKT)]
                wq_sb = [wpool.tile([128, D], F32R, tag=f"wq{k}", name=f"wq{k}") for k in range(NKT)]
                wk_sb = [wpool.tile([128, D], F32R, tag=f"wk{k}", name=f"wk{k}") for k in range(NKT)]
                wv_sb = [wpool.tile([128, D], F32R, tag=f"wv{k}", name=f"wv{k}") for k in range(NKT)]
                for k in range(NKT):
                    nc.sync.dma_start(wi_sb[k][:], wi[128 * k:128 * (k + 1), :].bitcast(F32R))
                    nc.sync.dma_start(wq_sb[k][:], wq[128 * k:128 * (k + 1), :].bitcast(F32R))
                    nc.sync.dma_start(wk_sb[k][:], wk[128 * k:128 * (k + 1), :].bitcast(F32R))
                    nc.sync.dma_start(wv_sb[k][:], wv[128 * k:128 * (k + 1), :].bitcast(F32R))

                for mc in range(NMC):
                    c0 = 512 * mc
                    xc = [apool.tile([128, 512], F32R, tag=f"xc{k}", bufs=1, name=f"xc{k}")
                          for k in range(NKT)]
                    for k in range(NKT):
                        nc.sync.dma_start(
                            xc[k][:], xT[128 * k:128 * (k + 1), c0:c0 + 512].bitcast(F32R))

                    # h0^T chunk [1024, 512] feature-major
                    h0 = [apool.tile([128, 512], F32R, tag=f"h0{n}", bufs=1, name=f"h0{n}")
                          for n in range(NKT)]
                    for n in range(NKT):
                        ph = pspool.tile([128, 512], F32, tag="ps")
                        for k in range(NKT):
                            nc.tensor.matmul(
                                ph[:], wi_sb[k][:, 128 * n:128 * (n + 1)], xc[k][:],
                                start=(k == 0), stop=(k == NKT - 1))
                        nc.vector.tensor_copy(h0[n][:], ph[:])

                    # q^T / k^T chunks (feature-major), spilled
                    for w_sb, b_ap, dst in (
                        (wq_sb, "bq", qT_d), (wk_sb, "bk", kT_d)):
                        for n in range(NKT):
                            pq = pspool.tile([128, 512], F32, tag="ps")
                            for k in range(NKT):
                                nc.tensor.matmul(
                                    pq[:], w_sb[k][:, 128 * n:128 * (n + 1)], h0[k][:],
                                    start=(k == 0), stop=(k == NKT - 1))
                            dst_ap = dst[128 * n:128 * (n + 1), c0:c0 + 512]
                            qs = apool.tile([128, 512], F32, tag="spill", bufs=4,
                                            name=f"qs{n}")
                            if with_qk_bias:
                                bias = (bq_sb if b_ap == "bq" else bk_sb)[:, n:n + 1]
                                nc.vector.tensor_scalar_add(qs[:], pq[:], bias)
                            else:
                                nc.vector.tensor_copy(qs[:], pq[:])
                            nc.sync.dma_start(dst_ap, qs[:])

                    # v chunk (token-major), spilled
                    for tt in range(4):
                        for nn in range(2):
                            pv = pspool.tile([128, 512], F32, tag="ps")
                            for k in range(NKT):
                                nc.tensor.matmul(
                                    pv[:], h0[k][:, 128 * tt:128 * (tt + 1)],
                                    wv_sb[k][:, 512 * nn:512 * (nn + 1)],
                                    start=(k == 0), stop=(k == NKT - 1))
                            vs = apool.tile([128, 512], F32, tag="spill", bufs=4,
                                            name=f"vs{tt}{nn}")
                            nc.vector.tensor_copy(vs[:], pv[:])
                            nc.sync.dma_start(
                                v_d[c0 + 128 * tt:c0 + 128 * (tt + 1),
                                    512 * nn:512 * (nn + 1)], vs[:])

            # ---------------- Phase B: attention + out_proj per batch --
            with tc.tile_pool(name="wB", bufs=1) as wpool, \
                 tc.tile_pool(name="actB", bufs=1) as apool, \
                 tc.tile_pool(name="psS", bufs=4, space="PSUM") as psS, \
                 tc.tile_pool(name="psO", bufs=2, space="PSUM") as psO, \
                 tc.tile_pool(name="psF", bufs=2, space="PSUM") as psF:
                wo_sb = [wpool.tile([128, D], F32R, tag=f"wo{k}", name=f"wo{k}") for k in range(NKT)]
                for k in range(NKT):
                    nc.sync.dma_start(wo_sb[k][:], wo[128 * k:128 * (k + 1), :].bitcast(F32R))

                for b in range(BN):
                    r0 = 512 * b
                    qt = [apool.tile([128, 512], F32R, tag=f"qt{e}", bufs=2, name=f"qt{e}")
                          for e in range(NKT)]
                    kt = [apool.tile([128, 512], F32R, tag=f"kt{e}", bufs=2, name=f"kt{e}")
                          for e in range(NKT)]
                    for e in range(NKT):
                        nc.sync.dma_start(
                            qt[e][:], qT_d[128 * e:128 * (e + 1), r0:r0 + 512].bitcast(F32R))
                        nc.sync.dma_start(
                            kt[e][:], kT_d[128 * e:128 * (e + 1), r0:r0 + 512].bitcast(F32R))
                    # v_plus tiles: [128, H, 65] = per-head 64 v cols + ones col
                    vp = [apool.tile([128, H * 65], F32R, tag=f"vp{i}", bufs=2, name=f"vp{i}")
                          for i in range(4)]
                    for i in range(4):
                        v3 = vp[i][:].rearrange("p (h e) -> p h e", e=65)
                        nc.sync.dma_start(
                            v3[:, :, 0:64],
                            v_d[r0 + 128 * i:r0 + 128 * (i + 1), :]
                            .bitcast(F32R).rearrange("p (h e) -> p h e", e=64))
                        nc.sync.dma_start(v3[:, :, 64], onesc[:, :].bitcast(F32R))

                    oT = [apool.tile([128, 512], F32R, tag=f"oT{e}", bufs=2, name=f"oT{e}")
                          for e in range(NKT)]
                    for j in range(H):
                        et, off = j // 2, 64 * (j % 2)
                        po = psO.tile([65, 512], F32, tag="po")
                        pt = [None] * 4
                        for i in range(4):
                            w0 = 128 * i  # valid t-cols are [w0, 512)
                            ps = psS.tile([128, 512], F32, tag="ps")
                            nc.tensor.matmul(
                                ps[:, w0:512],
                                qt[et][off:off + 64, w0:w0 + 128],
                                kt[et][off:off + 64, w0:512],
                                start=True, stop=True)
                            # causal mask on the diagonal block (s > t)
                            nc.vector.tensor_add(
                                ps[:, w0:w0 + 128], ps[:, w0:w0 + 128], tri_sb[:])
                            pt[i] = apool.tile([128, 512], F32R, tag="pt", bufs=8, name=f"pt{i}")
                            nc.scalar.activation(pt[i][:, w0:512], ps[:, w0:512], AF.Exp)
                        for i in range(4):
                            w0 = 128 * i
                            nc.tensor.matmul(
                                po[0:65, w0:512],
                                vp[i][:, 65 * j:65 * (j + 1)],
                                pt[i][:, w0:512],
                                start=(i == 0), stop=(i == 3), skip_group_check=True)
                        # normalize: O^T[e,t] / denom[t]
                        rs = apool.tile([1, 512], F32R, tag="rs", bufs=4)
                        with nc.allow_low_precision(reason="f32r softmax recip"):
                            nc.vector.reciprocal(rs[:], po[64:65, :])
                        rb = apool.tile([64, 512], F32R, tag="rb", bufs=4)
                        nc.gpsimd.partition_broadcast(rb[:], rs[:])
                        nc.vector.tensor_mul(oT[et][off:off + 64, :], po[0:64, :], rb[:])

                    # out_proj for this batch: out[t, n] (token-major)
                    for tt in range(4):
                        for nn in range(2):
                            pf = psF.tile([128, 512], F32, tag="pf")
                            for k in range(NKT):
                                nc.tensor.matmul(
                                    pf[:], oT[k][:, 128 * tt:128 * (tt + 1)],
                                    wo_sb[k][:, 512 * nn:512 * (nn + 1)],
                                    start=(k == 0), stop=(k == NKT - 1))
                            os_ = apool.tile([128, 512], F32, tag="os", bufs=3,
                                             name=f"os{tt}{nn}")
                            nc.vector.tensor_copy(os_[:], pf[:])
                            nc.sync.dma_start(
                                out[r0 + 128 * tt:r0 + 128 * (tt + 1),
                                    512 * nn:512 * (nn + 1)], os_[:])

    nc.compile()
    return nc


def kernel(x, Wi, bi, Wk, bk, Wq, bq, Wv, bv, Wo, bo):
    x, Wi, bi = np.asarray(x, np.float32), np.asarray(Wi, np.float32), np.asarray(bi, np.float32)
    Wk, bk = np.asarray(Wk, np.float32), np.asarray(bk, np.float32)
    Wq, bq = np.asarray(Wq, np.float32), np.asarray(bq, np.float32)
    Wv, bv = np.asarray(Wv, np.float32), np.asarray(bv, np.float32)
    Wo, bo = np.asarray(Wo, np.float32), np.asarray(bo, np.float32)

    # flatten head-stacked weights: col f = h*HD + e
    wq_f = np.ascontiguousarray(Wq.transpose(1, 0, 2).reshape(D, D))
    wk_f = np.ascontiguousarray(Wk.transpose(1, 0, 2).reshape(D, D))
    wv_f = np.ascontiguousarray(Wv.transpose(1, 0, 2).reshape(D, D))
    # fold bi through the qkv projections; fold bv through out_proj
    bq_fold = (bi @ wq_f + bq.reshape(-1)).astype(np.float32)
    bk_fold = (bi @ wk_f + bk.reshape(-1)).astype(np.float32)
    bv_fold = (bi @ wv_f + bv.reshape(-1)).astype(np.float32)
    out_const = (bv_fold @ Wo + bo).astype(np.float32)  # added host-side

    with_qk_bias = bool(np.any(bq_fold) or np.any(bk_fold))
    if with_qk_bias not in _CACHE:
        _CACHE[with_qk_bias] = _build(with_qk_bias)
    nc = _CACHE[with_qk_bias]

    tri_add = ((np.triu(np.ones((128, 128))) - 1.0) * -MASK_NEG).astype(np.float32)
    onesc = np.ones((128, H), np.float32)

    shared = {"wi": Wi, "wq": wq_f, "wk": wk_f, "wv": wv_f, "wo": Wo,
              "tri": tri_add, "onesc": onesc}
    if with_qk_bias:
        shared["bq2"] = np.ascontiguousarray(bq_fold.reshape(NKT, 128).T)
        shared["bk2"] = np.ascontiguousarray(bk_fold.reshape(NKT, 128).T)

    in_maps = []
    for c in range(NCORES):
        xs = x[BN * c:BN * (c + 1)].reshape(TOK, D)
        m = dict(shared)
        m["xT"] = np.ascontiguousarray(xs.T)
        in_maps.append(m)

    res = bass_utils.run_bass_kernel_spmd(nc, in_maps, core_ids=list(range(NCORES)))
    outs = [res.results[c]["out"] for c in range(NCORES)]
    full = np.concatenate(outs, axis=0).reshape(B, T, D)
    full += out_const[None, None, :]
    return full


# revision 8
# speedup vs baseline: 2326.8888x; 2326.8888x over previous
"""Trainium2 Bass kernel for the reference MultiHeadAttention module.

Problem: B=32, T=512, D=1024, H=16, HD=64 (see reference semantics below).

Reference computation (note the unusual orientation: keys index rows,
queries index softmax axis, no 1/sqrt(d) scale):
    h  = x @ Wi + bi
    k/q/v = per-head h @ W{k,q,v}[h] + b (head-stacked weights)
    wei[b,h,t,s] = k[b,h,t,:] . q[b,h,s,:]      (t = key idx, s = query idx)
    wei masked to s <= t, softmax over s
    out = (wei @ v) concat-heads @ Wo + bo

Sharding: data-parallel over batch. Each of the 8 cores processes 4
batches (2048 tokens) with replicated weights; no collectives. Host
re-assembles the full [32,512,1024] output.

Device dataflow per core (all matmuls in float32r: full-rate PE with
~13-bit mantissa operands, fp32 PSUM accumulation):
  Phase A (per 512-token chunk, streaming):
    h0^T = Wi^T-chain from host-transposed x^T (feature-major),
    q^T / k^T (feature-major) and v (token-major) from h0^T,
    spilled to DRAM scratch (SBUF can't hold Wi+Wqkv+activations).
  Phase B (per batch of 512 tokens):
    S^T[s,t] per head via q^T/k^T slices (base-partition 0/64),
    causal mask = additive -60000 on the diagonal 128x128 block plus
    column-range restriction (s-tile i only computes t >= 128*i),
    P^T = exp(S^T) on ACT (no max-subtraction: |logits| <= ~55 is safe
    in fp32 since the reference softmax is unscaled and unsubtracted
    errors cancel exactly in the ratio),
    O^T = [V | 1]^T @ P^T per head (ones column yields the softmax
    denominator as row 64), reciprocal + partition_broadcast + multiply
    to normalize, then out = O^T-chain @ Wo token-major, DMA'd out.

All biases are handled host-side or folded:
  bi,bk,bq fold into per-feature adds on q^T/k^T (applied on-device only
  if nonzero; they are zero in setup_inputs), bv+bi fold into a constant
  row added on host after out_proj, bo added on host.
"""

import sys

sys.path.insert(0, "/opt/trn_rl_repo")

import numpy as np

import concourse.bacc as bacc
import concourse.mybir as mybir
from concourse import bass_utils
from concourse.tile import TileContext

F32 = mybir.dt.float32
F32R = mybir.dt.float32r
AF = mybir.ActivationFunctionType

B, T, D, H, HD = 32, 512, 1024, 16, 64
NCORES = 8
BN = B // NCORES          # batches per core = 4
TOK = BN * T              # tokens per core = 2048
NKT = D // 128            # 8 contraction tiles
NMC = TOK // 512          # 4 token chunks (phase A)
MASK_NEG = -60000.0       # exp(-60000 + |logit|) == 0 in fp32

_CACHE = {}


def _build(with_qk_bias: bool):
    nc = bacc.Bacc("TRN2", target_bir_lowering=False, debug=False,
                   num_devices=NCORES)

    xT = nc.dram_tensor("xT", [D, TOK], F32, kind="ExternalInput")
    wi = nc.dram_tensor("wi", [D, D], F32, kind="ExternalInput")
    wq = nc.dram_tensor("wq", [D, D], F32, kind="ExternalInput")
    wk = nc.dram_tensor("wk", [D, D], F32, kind="ExternalInput")
    wv = nc.dram_tensor("wv", [D, D], F32, kind="ExternalInput")
    wo = nc.dram_tensor("wo", [D, D], F32, kind="ExternalInput")
    tri = nc.dram_tensor("tri", [128, 128], F32, kind="ExternalInput")
    onesc = nc.dram_tensor("onesc", [128, H], F32, kind="ExternalInput")
    if with_qk_bias:
        bq2 = nc.dram_tensor("bq2", [128, NKT], F32, kind="ExternalInput")
        bk2 = nc.dram_tensor("bk2", [128, NKT], F32, kind="ExternalInput")
    out = nc.dram_tensor("out", [TOK, D], F32, kind="ExternalOutput")

    # DRAM scratch spills (feature-major q/k, token-major v)
    qT_d = nc.dram_tensor("qT_d", [D, TOK], F32, kind="Internal")
    kT_d = nc.dram_tensor("kT_d", [D, TOK], F32, kind="Internal")
    v_d = nc.dram_tensor("v_d", [TOK, D], F32, kind="Internal")

    with TileContext(nc) as tc:
        with tc.tile_pool(name="const", bufs=1) as cpool:
            tri_sb = cpool.tile([128, 128], F32, tag="tri")
            nc.sync.dma_start(tri_sb[:], tri[:])
            if with_qk_bias:
                bq_sb = cpool.tile([128, NKT], F32, tag="bq")
                bk_sb = cpool.tile([128, NKT], F32, tag="bk")
                nc.sync.dma_start(bq_sb[:], bq2[:])
                nc.sync.dma_start(bk_sb[:], bk2[:])

            # ---------------- Phase A: in_proj + QKV, spill to DRAM ----
            with tc.tile_pool(name="wA", bufs=1) as wpool, \
                 tc.tile_pool(name="actA", bufs=1) as apool, \
                 tc.tile_pool(name="psA", bufs=7, space="PSUM") as pspool:
                wi_sb = [wpool.tile([128, D], F32R, tag=f"wi{k}", name=f"wi{k}") for k in range(NKT)]
                wq_sb = [wpool.tile([128, D], F32R, tag=f"wq{k}", name=f"wq{k}") for k in range(NKT)]
                wk_sb = [wpool.tile([128, D], F32R, tag=f"wk{k}", name=f"wk{k}") for k in range(NKT)]
                wv_sb = [wpool.tile([128, D], F32R, tag=f"wv{k}", name=f"wv{k}") for k in range(NKT)]
                for k in range(NKT):
                    nc.sync.dma_start(wi_sb[k][:], wi[128 * k:128 * (k + 1), :].bitcast(F32R))
                    nc.sync.dma_start(wq_sb[k][:], wq[128 * k:128 * (k + 1), :].bitcast(F32R))
                    nc.sync.dma_start(wk_sb[k][:], wk[128 * k:128 * (k + 1), :].bitcast(F32R))
                    nc.sync.dma_start(wv_sb[k][:], wv[128 * k:128 * (k + 1), :].bitcast(F32R))

                for mc in range(NMC):
                    c0 = 512 * mc
                    xc = [apool.tile([128, 512], F32R, tag=f"xc{k}", bufs=1, name=f"xc{k}")
                          for k in range(NKT)]
                    for k in range(NKT):
                        nc.sync.dma_start(
                            xc[k][:], xT[128 * k:128 * (k + 1), c0:c0 + 512].bitcast(F32R))

                    # h0^T chunk [1024, 512] feature-major
                    h0 = [apool.tile([128, 512], F32R, tag=f"h0{n}", bufs=1, name=f"h0{n}")
                          for n in range(NKT)]
                    for n in range(NKT):
                        ph = pspool.tile([128, 512], F32, tag="ps")
                        for k in range(NKT):
                            nc.tensor.matmul(
                                ph[:], wi_sb[k][:, 128 * n:128 * (n + 1)], xc[k][:],
                                start=(k == 0), stop=(k == NKT - 1))
                        nc.vector.tensor_copy(h0[n][:], ph[:])

                    # q^T / k^T chunks (feature-major), spilled
                    for w_sb, b_ap, dst in (
                        (wq_sb, "bq", qT_d), (wk_sb, "bk", kT_d)):
                        for n in range(NKT):
                            pq = pspool.tile([128, 512], F32, tag="ps")
                            for k in range(NKT):
                                nc.tensor.matmul(
                                    pq[:], w_sb[k][:, 128 * n:128 * (n + 1)], h0[k][:],
                                    start=(k == 0), stop=(k == NKT - 1))
                            dst_ap = dst[128 * n:128 * (n + 1), c0:c0 + 512]
                            qs = apool.tile([128, 512], F32, tag="spill", bufs=4,
                                            name=f"qs{n}")
                            if with_qk_bias:
                                bias = (bq_sb if b_ap == "bq" else bk_sb)[:, n:n + 1]
                                nc.vector.tensor_scalar_add(qs[:], pq[:], bias)
                            else:
                                nc.vector.tensor_copy(qs[:], pq[:])
                            nc.sync.dma_start(dst_ap, qs[:])

                    # v chunk (token-major), spilled
                    for tt in range(4):
                        for nn in range(2):
                            pv = pspool.tile([128, 512], F32, tag="ps")
                            for k in range(NKT):
                                nc.tensor.matmul(
                                    pv[:], h0[k][:, 128 * tt:128 * (tt + 1)],
                                    wv_sb[k][:, 512 * nn:512 * (nn + 1)],
                                    start=(k == 0), stop=(k == NKT - 1))
                            vs = apool.tile([128, 512], F32, tag="spill", bufs=4,
                                            name=f"vs{tt}{nn}")
                            nc.vector.tensor_copy(vs[:], pv[:])
                            nc.sync.dma_start(
                                v_d[c0 + 128 * tt:c0 + 128 * (tt + 1),
                                    512 * nn:512 * (nn + 1)], vs[:])

            # ---------------- Phase B: attention + out_proj per batch --
            with tc.tile_pool(name="wB", bufs=1) as wpool, \
                 tc.tile_pool(name="actB", bufs=1) as apool, \
                 tc.tile_pool(name="psS", bufs=4, space="PSUM") as psS, \
                 tc.tile_pool(name="psO", bufs=2, space="PSUM") as psO, \
                 tc.tile_pool(name="psF", bufs=2, space="PSUM") as psF:
                wo_sb = [wpool.tile([128, D], F32R, tag=f"wo{k}", name=f"wo{k}") for k in range(NKT)]
                for k in range(NKT):
                    nc.sync.dma_start(wo_sb[k][:], wo[128 * k:128 * (k + 1), :].bitcast(F32R))

                for b in range(BN):
                    r0 = 512 * b
                    qt = [apool.tile([128, 512], F32R, tag=f"qt{e}", bufs=2, name=f"qt{e}")
                          for e in range(NKT)]
                    kt = [apool.tile([128, 512], F32R, tag=f"kt{e}", bufs=2, name=f"kt{e}")
                          for e in range(NKT)]
                    for e in range(NKT):
                        nc.sync.dma_start(
                            qt[e][:], qT_d[128 * e:128 * (e + 1), r0:r0 + 512].bitcast(F32R))
                        nc.sync.dma_start(
                            kt[e][:], kT_d[128 * e:128 * (e + 1), r0:r0 + 512].bitcast(F32R))
                    # v_plus tiles: [128, H, 65] = per-head 64 v cols + ones col
                    vp = [apool.tile([128, H * 65], F32R, tag=f"vp{i}", bufs=2, name=f"vp{i}")
                          for i in range(4)]
                    for i in range(4):
                        v3 = vp[i][:].rearrange("p (h e) -> p h e", e=65)
                        nc.sync.dma_start(
                            v3[:, :, 0:64],
                            v_d[r0 + 128 * i:r0 + 128 * (i + 1), :]
                            .bitcast(F32R).rearrange("p (h e) -> p h e", e=64))
                        nc.sync.dma_start(v3[:, :, 64], onesc[:, :].bitcast(F32R))

                    oT = [apool.tile([128, 512], F32R, tag=f"oT{e}", bufs=2, name=f"oT{e}")
                          for e in range(NKT)]
                    for j in range(H):
                        et, off = j // 2, 64 * (j % 2)
                        po = psO.tile([65, 512], F32, tag="po")
                        pt = [None] * 4
                        for i in range(4):
                            w0 = 128 * i  # valid t-cols are [w0, 512)
                            ps = psS.tile([128, 512], F32, tag="ps")
                            nc.tensor.matmul(
                                ps[:, w0:512],
                                qt[et][off:off + 64, w0:w0 + 128],
                                kt[et][off:off + 64, w0:512],
                                start=True, stop=True)
                            # causal mask on the diagonal block (s > t)
                            nc.vector.tensor_add(
                                ps[:, w0:w0 + 128], ps[:, w0:w0 + 128], tri_sb[:])
                            pt[i] = apool.tile([128, 512], F32R, tag="pt", bufs=8, name=f"pt{i}")
                            nc.scalar.activation(pt[i][:, w0:512], ps[:, w0:512], AF.Exp)
                        for i in range(4):
                            w0 = 128 * i
                            nc.tensor.matmul(
                                po[0:65, w0:512],
                                vp[i][:, 65 * j:65 * (j + 1)],
                                pt[i][:, w0:512],
                                start=(i == 0), stop=(i == 3), skip_group_check=True)
                        # normalize: O^T[e,t] / denom[t]
                        rs = apool.tile([1, 512], F32R, tag="rs", bufs=4)
                        with nc.allow_low_precision(reason="f32r softmax recip"):
                            nc.vector.reciprocal(rs[:], po[64:65, :])
                        rb = apool.tile([64, 512], F32R, tag="rb", bufs=4)
                        nc.gpsimd.partition_broadcast(rb[:], rs[:])
                        nc.vector.tensor_mul(oT[et][off:off + 64, :], po[0:64, :], rb[:])

                    # out_proj for this batch: out[t, n] (token-major)
                    for tt in range(4):
                        for nn in range(2):
                            pf = psF.tile([128, 512], F32, tag="pf")
                            for k in range(NKT):
                                nc.tensor.matmul(
                                    pf[:], oT[k][:, 128 * tt:128 * (tt + 1)],
                                    wo_sb[k][:, 512 * nn:512 * (nn + 1)],
                                    start=(k == 0), stop=(k == NKT - 1))
                            os_ = apool.tile([128, 512], F32, tag="os", bufs=3,
                                             name=f"os{tt}{nn}")
                            nc.vector.tensor_copy(os_[:], pf[:])
                            nc.sync.dma_start(
                                out[r0 + 128 * tt:r0 + 128 * (tt + 1),
                                    512 * nn:512 * (nn + 1)], os_[:])

    nc.compile()
    return nc


def _ensure_built(with_qk_bias: bool):
    if with_qk_bias not in _CACHE:
        _CACHE[with_qk_bias] = _build(with_qk_bias)
    return _CACHE[with_qk_bias]


def _prepare(x, Wi, bi, Wk, bk, Wq, bq, Wv, bv, Wo, bo):
    """Host-side prep: returns (in_maps, out_const, with_qk_bias)."""
    x, Wi, bi = np.asarray(x, np.float32), np.asarray(Wi, np.float32), np.asarray(bi, np.float32)
    Wk, bk = np.asarray(Wk, np.float32), np.asarray(bk, np.float32)
    Wq, bq = np.asarray(Wq, np.float32), np.asarray(bq, np.float32)
    Wv, bv = np.asarray(Wv, np.float32), np.asarray(bv, np.float32)
    Wo, bo = np.asarray(Wo, np.float32), np.asarray(bo, np.float32)

    # flatten head-stacked weights: col f = h*HD + e
    wq_f = np.ascontiguousarray(Wq.transpose(1, 0, 2).reshape(D, D))
    wk_f = np.ascontiguousarray(Wk.transpose(1, 0, 2).reshape(D, D))
    wv_f = np.ascontiguousarray(Wv.transpose(1, 0, 2).reshape(D, D))
    # fold bi through the qkv projections; fold bv through out_proj
    bq_fold = (bi @ wq_f + bq.reshape(-1)).astype(np.float32)
    bk_fold = (bi @ wk_f + bk.reshape(-1)).astype(np.float32)
    bv_fold = (bi @ wv_f + bv.reshape(-1)).astype(np.float32)
    out_const = (bv_fold @ Wo + bo).astype(np.float32)  # added host-side

    with_qk_bias = bool(np.any(bq_fold) or np.any(bk_fold))

    tri_add = ((np.triu(np.ones((128, 128))) - 1.0) * -MASK_NEG).astype(np.float32)
    onesc = np.ones((128, H), np.float32)

    shared = {"wi": Wi, "wq": wq_f, "wk": wk_f, "wv": wv_f, "wo": Wo,
              "tri": tri_add, "onesc": onesc}
    if with_qk_bias:
        shared["bq2"] = np.ascontiguousarray(bq_fold.reshape(NKT, 128).T)
        shared["bk2"] = np.ascontiguousarray(bk_fold.reshape(NKT, 128).T)

    in_maps = []
    for c in range(NCORES):
        xs = x[BN * c:BN * (c + 1)].reshape(TOK, D)
        m = dict(shared)
        m["xT"] = np.ascontiguousarray(xs.T)
        in_maps.append(m)
    return in_maps, out_const, with_qk_bias


def kernel(x, Wi, bi, Wk, bk, Wq, bq, Wv, bv, Wo, bo):
    in_maps, out_const, with_qk_bias = _prepare(
        x, Wi, bi, Wk, bk, Wq, bq, Wv, bv, Wo, bo)
    nc = _ensure_built(with_qk_bias)
    res = bass_utils.run_bass_kernel_spmd(nc, in_maps, core_ids=list(range(NCORES)))
    outs = [res.results[c]["out"] for c in range(NCORES)]
    full = np.concatenate(outs, axis=0).reshape(B, T, D)
    full += out_const[None, None, :]
    return full


# revision 15
# speedup vs baseline: 2374.1828x; 1.0203x over previous
"""Trainium2 Bass kernel for the reference MultiHeadAttention module.

Problem: B=32, T=512, D=1024, H=16, HD=64 (see reference semantics below).

Reference computation (note the unusual orientation: keys index rows,
queries index softmax axis, no 1/sqrt(d) scale):
    h  = x @ Wi + bi
    k/q/v = per-head h @ W{k,q,v}[h] + b (head-stacked weights)
    wei[b,h,t,s] = k[b,h,t,:] . q[b,h,s,:]      (t = key idx, s = query idx)
    wei masked to s <= t, softmax over s
    out = (wei @ v) concat-heads @ Wo + bo

Sharding: data-parallel over batch. Each of the 8 cores processes 4
batches (2048 tokens) with replicated weights; no collectives. Host
re-assembles the full [32,512,1024] output.

Device dataflow per core (all matmuls in float32r: full-rate PE with
~13-bit mantissa operands, fp32 PSUM accumulation):
  Phase A (per 512-token chunk, streaming):
    h0^T = Wi^T-chain from host-transposed x^T (feature-major),
    q^T / k^T (feature-major) and v (token-major) from h0^T,
    spilled to DRAM scratch (SBUF can't hold Wi+Wqkv+activations).
  Phase B (per batch of 512 tokens):
    S^T[s,t] per head via q^T/k^T slices (base-partition 0/64),
    causal mask = additive -60000 on the diagonal 128x128 block plus
    column-range restriction (s-tile i only computes t >= 128*i),
    P^T = exp(S^T) on ACT (no max-subtraction: |logits| <= ~55 is safe
    in fp32 since the reference softmax is unscaled and unsubtracted
    errors cancel exactly in the ratio),
    O^T = [V | 1]^T @ P^T per head (ones column yields the softmax
    denominator as row 64), reciprocal + partition_broadcast + multiply
    to normalize, then out = O^T-chain @ Wo token-major, DMA'd out.

All biases are handled host-side or folded:
  bi,bk,bq fold into per-feature adds on q^T/k^T (applied on-device only
  if nonzero; they are zero in setup_inputs), bv+bi fold into a constant
  row added on host after out_proj, bo added on host.
"""

import sys

sys.path.insert(0, "/opt/trn_rl_repo")

import numpy as np

import concourse.bacc as bacc
import concourse.mybir as mybir
from concourse import bass_utils
from concourse.tile import TileContext

F32 = mybir.dt.float32
F32R = mybir.dt.float32r
AF = mybir.ActivationFunctionType

B, T, D, H, HD = 32, 512, 1024, 16, 64
NCORES = 8
BN = B // NCORES          # batches per core = 4
TOK = BN * T              # tokens per core = 2048
NKT = D // 128            # 8 contraction tiles
NMC = TOK // 512          # 4 token chunks (phase A)
MASK_NEG = -60000.0       # exp(-60000 + |logit|) == 0 in fp32

_CACHE = {}


def _build(with_qk_bias: bool):
    nc = bacc.Bacc("TRN2", target_bir_lowering=False, debug=False,
                   num_devices=NCORES)

    xT = nc.dram_tensor("xT", [D, TOK], F32, kind="ExternalInput")
    wi = nc.dram_tensor("wi", [D, D], F32, kind="ExternalInput")
    wq = nc.dram_tensor("wq", [D, D], F32, kind="ExternalInput")
    wk = nc.dram_tensor("wk", [D, D], F32, kind="ExternalInput")
    wv = nc.dram_tensor("wv", [D, D], F32, kind="ExternalInput")
    wo = nc.dram_tensor("wo", [D, D], F32, kind="ExternalInput")
    tri = nc.dram_tensor("tri", [128, 128], F32, kind="ExternalInput")
    onesc = nc.dram_tensor("onesc", [128, H], F32, kind="ExternalInput")
    if with_qk_bias:
        bq2 = nc.dram_tensor("bq2", [128, NKT], F32, kind="ExternalInput")
        bk2 = nc.dram_tensor("bk2", [128, NKT], F32, kind="ExternalInput")
    out = nc.dram_tensor("out", [TOK, D], F32, kind="ExternalOutput")

    # DRAM scratch spills (feature-major q/k, token-major v)
    qT_d = nc.dram_tensor("qT_d", [D, TOK], F32, kind="Internal")
    kT_d = nc.dram_tensor("kT_d", [D, TOK], F32, kind="Internal")
    v_d = nc.dram_tensor("v_d", [TOK, D], F32, kind="Internal")

    with TileContext(nc) as tc:
        with tc.tile_pool(name="const", bufs=1) as cpool:
            tri_sb = cpool.tile([128, 128], F32, tag="tri")
            nc.sync.dma_start(tri_sb[:], tri[:])
            if with_qk_bias:
                bq_sb = cpool.tile([128, NKT], F32, tag="bq")
                bk_sb = cpool.tile([128, NKT], F32, tag="bk")
                nc.sync.dma_start(bq_sb[:], bq2[:])
                nc.sync.dma_start(bk_sb[:], bk2[:])

            # ---------------- Phase A: in_proj + QKV, spill to DRAM ----
            with tc.tile_pool(name="wA", bufs=1) as wpool, \
                 tc.tile_pool(name="actA", bufs=1) as apool, \
                 tc.tile_pool(name="psA", bufs=7, space="PSUM") as pspool:
                wi_sb = [wpool.tile([128, D], F32R, tag=f"wi{k}", name=f"wi{k}") for k in range(NKT)]
                wq_sb = [wpool.tile([128, D], F32R, tag=f"wq{k}", name=f"wq{k}") for k in range(NKT)]
                wk_sb = [wpool.tile([128, D], F32R, tag=f"wk{k}", name=f"wk{k}") for k in range(NKT)]
                wv_sb = [wpool.tile([128, D], F32R, tag=f"wv{k}", name=f"wv{k}") for k in range(NKT)]
                # DMA order matters for the startup critical path: the first
                # chunk only needs Wi + x, so issue those first and let the
                # 12 MiB of Wq/Wk/Wv stream in under chunk-0's in_proj.
                xc0 = [apool.tile([128, 512], F32R, tag=f"xc{k}", bufs=1, name=f"xc{k}")
                       for k in range(NKT)]
                for k in range(NKT):
                    nc.sync.dma_start(wi_sb[k][:], wi[128 * k:128 * (k + 1), :].bitcast(F32R))
                    nc.sync.dma_start(
                        xc0[k][:], xT[128 * k:128 * (k + 1), 0:512].bitcast(F32R))

                for mc in range(NMC):
                    c0 = 512 * mc
                    if mc == 0:
                        xc = xc0
                    else:
                        xc = [apool.tile([128, 512], F32R, tag=f"xc{k}", bufs=1, name=f"xc{k}")
                              for k in range(NKT)]
                        for k in range(NKT):
                            nc.sync.dma_start(
                                xc[k][:], xT[128 * k:128 * (k + 1), c0:c0 + 512].bitcast(F32R))
                    if mc == 0:
                        for k in range(NKT):
                            nc.sync.dma_start(wq_sb[k][:], wq[128 * k:128 * (k + 1), :].bitcast(F32R))
                            nc.sync.dma_start(wk_sb[k][:], wk[128 * k:128 * (k + 1), :].bitcast(F32R))
                            nc.sync.dma_start(wv_sb[k][:], wv[128 * k:128 * (k + 1), :].bitcast(F32R))

                    # h0^T chunk [1024, 512] feature-major
                    h0 = [apool.tile([128, 512], F32R, tag=f"h0{n}", bufs=1, name=f"h0{n}")
                          for n in range(NKT)]
                    for n in range(NKT):
                        ph = pspool.tile([128, 512], F32, tag="ps")
                        for k in range(NKT):
                            nc.tensor.matmul(
                                ph[:], wi_sb[k][:, 128 * n:128 * (n + 1)], xc[k][:],
                                start=(k == 0), stop=(k == NKT - 1))
                        nc.vector.tensor_copy(h0[n][:], ph[:])

                    # q^T / k^T chunks (feature-major), spilled
                    for w_sb, b_ap, dst in (
                        (wq_sb, "bq", qT_d), (wk_sb, "bk", kT_d)):
                        for n in range(NKT):
                            pq = pspool.tile([128, 512], F32, tag="ps")
                            for k in range(NKT):
                                nc.tensor.matmul(
                                    pq[:], w_sb[k][:, 128 * n:128 * (n + 1)], h0[k][:],
                                    start=(k == 0), stop=(k == NKT - 1))
                            dst_ap = dst[128 * n:128 * (n + 1), c0:c0 + 512]
                            qs = apool.tile([128, 512], F32, tag="spill", bufs=4,
                                            name=f"qs{n}")
                            if with_qk_bias:
                                bias = (bq_sb if b_ap == "bq" else bk_sb)[:, n:n + 1]
                                nc.vector.tensor_scalar_add(qs[:], pq[:], bias)
                            else:
                                nc.vector.tensor_copy(qs[:], pq[:])
                            nc.sync.dma_start(dst_ap, qs[:])

                    # v chunk (token-major), spilled
                    for tt in range(4):
                        for nn in range(2):
                            pv = pspool.tile([128, 512], F32, tag="ps")
                            for k in range(NKT):
                                nc.tensor.matmul(
                                    pv[:], h0[k][:, 128 * tt:128 * (tt + 1)],
                                    wv_sb[k][:, 512 * nn:512 * (nn + 1)],
                                    start=(k == 0), stop=(k == NKT - 1))
                            vs = apool.tile([128, 512], F32, tag="spill", bufs=4,
                                            name=f"vs{tt}{nn}")
                            nc.vector.tensor_copy(vs[:], pv[:])
                            nc.sync.dma_start(
                                v_d[c0 + 128 * tt:c0 + 128 * (tt + 1),
                                    512 * nn:512 * (nn + 1)], vs[:])

            # ---------------- Phase B: attention + out_proj per batch --
            with tc.tile_pool(name="wB", bufs=1) as wpool, \
                 tc.tile_pool(name="actB", bufs=1) as apool, \
                 tc.tile_pool(name="psS", bufs=5, space="PSUM") as psS, \
                 tc.tile_pool(name="psO", bufs=2, space="PSUM") as psO, \
                 tc.tile_pool(name="psF", bufs=1, space="PSUM") as psF:
                wo_sb = [wpool.tile([128, D], F32R, tag=f"wo{k}", name=f"wo{k}") for k in range(NKT)]
                wo_loaded = [False]

                def load_wo():
                    for k in range(NKT):
                        nc.sync.dma_start(wo_sb[k][:], wo[128 * k:128 * (k + 1), :].bitcast(F32R))
                    wo_loaded[0] = True

                def attention(b):
                    """Emit qkv loads + 16 heads of attention for batch b.
                    Returns the normalized oT tiles."""
                    r0 = 512 * b
                    qt = [apool.tile([128, 512], F32R, tag=f"qt{e}", bufs=2, name=f"qt{e}")
                          for e in range(NKT)]
                    kt = [apool.tile([128, 512], F32R, tag=f"kt{e}", bufs=2, name=f"kt{e}")
                          for e in range(NKT)]
                    for e in range(NKT):
                        nc.sync.dma_start(
                            qt[e][:], qT_d[128 * e:128 * (e + 1), r0:r0 + 512].bitcast(F32R))
                        nc.sync.dma_start(
                            kt[e][:], kT_d[128 * e:128 * (e + 1), r0:r0 + 512].bitcast(F32R))
                    # v_plus tiles: [128, H, 65] = per-head 64 v cols + ones col
                    vp = [apool.tile([128, H * 65], F32R, tag=f"vp{i}", bufs=2, name=f"vp{i}")
                          for i in range(4)]
                    for i in range(4):
                        v3 = vp[i][:].rearrange("p (h e) -> p h e", e=65)
                        nc.sync.dma_start(
                            v3[:, :, 0:64],
                            v_d[r0 + 128 * i:r0 + 128 * (i + 1), :]
                            .bitcast(F32R).rearrange("p (h e) -> p h e", e=64))
                        nc.sync.dma_start(v3[:, :, 64], onesc[:, :].bitcast(F32R))
                    if not wo_loaded[0]:
                        load_wo()

                    oT = [apool.tile([128, 512], F32R, tag=f"oT{e}", bufs=2, name=f"oT{e}")
                          for e in range(NKT)]
                    # process heads in base-partition pairs (rows 0-63 /
                    # 64-127 of the same e-tile -> distinct PE row groups)
                    for m in range(H // 2):
                        et = m
                        pos = {}
                        pts = {}
                        for i in range(4):
                            w0 = 128 * i  # valid t-cols are [w0, 512)
                            for j in (2 * m, 2 * m + 1):
                                off = 64 * (j % 2)
                                ps = psS.tile([128, 512], F32, tag="ps",
                                              name=f"ps{i}{j % 2}")
                                nc.tensor.matmul(
                                    ps[:, w0:512],
                                    qt[et][off:off + 64, w0:w0 + 128],
                                    kt[et][off:off + 64, w0:512],
                                    start=True, stop=True)
                                # causal mask on the diagonal block (s > t)
                                nc.vector.tensor_add(
                                    ps[:, w0:w0 + 128], ps[:, w0:w0 + 128], tri_sb[:])
                                pt = apool.tile([128, 512], F32R, tag="pt", bufs=8,
                                                name=f"pt{i}{j % 2}")
                                nc.scalar.activation(pt[:, w0:512], ps[:, w0:512], AF.Exp)
                                pts[(j, i)] = pt
                        for j in (2 * m, 2 * m + 1):
                            po = psO.tile([65, 512], F32, tag="po", name=f"po{j % 2}")
                            pos[j] = po
                            for i in range(4):
                                w0 = 128 * i
                                nc.tensor.matmul(
                                    po[0:65, w0:512],
                                    vp[i][:, 65 * j:65 * (j + 1)],
                                    pts[(j, i)][:, w0:512],
                                    start=(i == 0), stop=(i == 3), skip_group_check=True)
                        # normalize the pair: O^T[e,t] / denom[t].
                        # partition_broadcast reads absolute partition 0, so
                        # each head's reciprocal lives in its own tile.
                        for j in (2 * m, 2 * m + 1):
                            off = 64 * (j % 2)
                            rs = apool.tile([1, 512], F32R, tag="rs", bufs=4, name="rs")
                            with nc.allow_low_precision(reason="f32r softmax recip"):
                                nc.vector.reciprocal(rs[:], pos[j][64:65, :])
                            rb = apool.tile([64, 512], F32R, tag="rb", bufs=4, name="rb")
                            nc.gpsimd.partition_broadcast(rb[:], rs[:])
                            nc.vector.tensor_mul(oT[et][off:off + 64, :],
                                                 pos[j][0:64, :], rb[:])
                    return oT

                def out_proj(b, oT):
                    """out[t, n] = oT-chain @ Wo for batch b (token-major)."""
                    r0 = 512 * b
                    for tt in range(4):
                        for nn in range(2):
                            pf = psF.tile([128, 512], F32, tag="pf")
                            for k in range(NKT):
                                nc.tensor.matmul(
                                    pf[:], oT[k][:, 128 * tt:128 * (tt + 1)],
                                    wo_sb[k][:, 512 * nn:512 * (nn + 1)],
                                    start=(k == 0), stop=(k == NKT - 1))
                            os_ = apool.tile([128, 512], F32, tag="os", bufs=3,
                                             name=f"os{tt}{nn}")
                            nc.vector.tensor_copy(os_[:], pf[:])
                            nc.sync.dma_start(
                                out[r0 + 128 * tt:r0 + 128 * (tt + 1),
                                    512 * nn:512 * (nn + 1)], os_[:])

                # interleave: attention(b+1) is emitted before out_proj(b) so
                # PE always has ready matmuls while batch b's normalization
                # tail resolves.
                prev = (0, attention(0))
                for b in range(1, BN):
                    cur = (b, attention(b))
                    out_proj(*prev)
                    prev = cur
                out_proj(*prev)

    nc.compile()
    return nc


def _ensure_built(with_qk_bias: bool):
    if with_qk_bias not in _CACHE:
        _CACHE[with_qk_bias] = _build(with_qk_bias)
    return _CACHE[with_qk_bias]


def _prepare(x, Wi, bi, Wk, bk, Wq, bq, Wv, bv, Wo, bo):
    """Host-side prep: returns (in_maps, out_const, with_qk_bias)."""
    x, Wi, bi = np.asarray(x, np.float32), np.asarray(Wi, np.float32), np.asarray(bi, np.float32)
    Wk, bk = np.asarray(Wk, np.float32), np.asarray(bk, np.float32)
    Wq, bq = np.asarray(Wq, np.float32), np.asarray(bq, np.float32)
    Wv, bv = np.asarray(Wv, np.float32), np.asarray(bv, np.float32)
    Wo, bo = np.asarray(Wo, np.float32), np.asarray(bo, np.float32)

    # flatten head-stacked weights: col f = h*HD + e
    wq_f = np.ascontiguousarray(Wq.transpose(1, 0, 2).reshape(D, D))
    wk_f = np.ascontiguousarray(Wk.transpose(1, 0, 2).reshape(D, D))
    wv_f = np.ascontiguousarray(Wv.transpose(1, 0, 2).reshape(D, D))
    # fold bi through the qkv projections; fold bv through out_proj
    bq_fold = (bi @ wq_f + bq.reshape(-1)).astype(np.float32)
    bk_fold = (bi @ wk_f + bk.reshape(-1)).astype(np.float32)
    bv_fold = (bi @ wv_f + bv.reshape(-1)).astype(np.float32)
    out_const = (bv_fold @ Wo + bo).astype(np.float32)  # added host-side

    with_qk_bias = bool(np.any(bq_fold) or np.any(bk_fold))

    tri_add = ((np.triu(np.ones((128, 128))) - 1.0) * -MASK_NEG).astype(np.float32)
    onesc = np.ones((128, H), np.float32)

    shared = {"wi": Wi, "wq": wq_f, "wk": wk_f, "wv": wv_f, "wo": Wo,
              "tri": tri_add, "onesc": onesc}
    if with_qk_bias:
        shared["bq2"] = np.ascontiguousarray(bq_fold.reshape(NKT, 128).T)
        shared["bk2"] = np.ascontiguousarray(bk_fold.reshape(NKT, 128).T)

    in_maps = []
    for c in range(NCORES):
        xs = x[BN * c:BN * (c + 1)].reshape(TOK, D)
        m = dict(shared)
        m["xT"] = np.ascontiguousarray(xs.T)
        in_maps.append(m)
    return in_maps, out_const, with_qk_bias


def kernel(x, Wi, bi, Wk, bk, Wq, bq, Wv, bv, Wo, bo):
    in_maps, out_const, with_qk_bias = _prepare(
        x, Wi, bi, Wk, bk, Wq, bq, Wv, bv, Wo, bo)
    nc = _ensure_built(with_qk_bias)
    res = bass_utils.run_bass_kernel_spmd(nc, in_maps, core_ids=list(range(NCORES)))
    outs = [res.results[c]["out"] for c in range(NCORES)]
    full = np.concatenate(outs, axis=0).reshape(B, T, D)
    full += out_const[None, None, :]
    return full


# revision 21
# speedup vs baseline: 2816.8659x; 1.1865x over previous
"""Trainium2 Bass kernel for the reference MultiHeadAttention module.

Problem: B=32, T=512, D=1024, H=16, HD=64 (see reference semantics below).

Reference computation (note the unusual orientation: keys index rows,
queries index softmax axis, no 1/sqrt(d) scale):
    h  = x @ Wi + bi
    k/q/v = per-head h @ W{k,q,v}[h] + b (head-stacked weights)
    wei[b,h,t,s] = k[b,h,t,:] . q[b,h,s,:]      (t = key idx, s = query idx)
    wei masked to s <= t, softmax over s
    out = (wei @ v) concat-heads @ Wo + bo

Sharding: data-parallel over batch. Each of the 8 cores processes 4
batches (2048 tokens) with replicated weights; no collectives. Host
re-assembles the full [32,512,1024] output.

Device dataflow per core (all matmuls in float32r: full-rate PE with
~13-bit mantissa operands, fp32 PSUM accumulation):
  Phase A (per 512-token chunk, streaming):
    h0^T = Wi^T-chain from host-transposed x^T (feature-major),
    q^T / k^T (feature-major) and v (token-major) from h0^T,
    spilled to DRAM scratch (SBUF can't hold Wi+Wqkv+activations).
  Phase B (per batch of 512 tokens):
    S^T[s,t] per head via q^T/k^T slices (base-partition 0/64),
    causal mask = additive -60000 on the diagonal 128x128 block plus
    column-range restriction (s-tile i only computes t >= 128*i),
    P^T = exp(S^T) on ACT (no max-subtraction: |logits| <= ~55 is safe
    in fp32 since the reference softmax is unscaled and unsubtracted
    errors cancel exactly in the ratio),
    O^T = [V | 1]^T @ P^T per head (ones column yields the softmax
    denominator as row 64), reciprocal + partition_broadcast + multiply
    to normalize, then out = O^T-chain @ Wo token-major, DMA'd out.

All biases are handled host-side or folded:
  bi,bk,bq fold into per-feature adds on q^T/k^T (applied on-device only
  if nonzero; they are zero in setup_inputs), bv+bi fold into a constant
  row added on host after out_proj, bo added on host.
"""

import sys

sys.path.insert(0, "/opt/trn_rl_repo")

import numpy as np

import concourse.bacc as bacc
import concourse.mybir as mybir
from concourse import bass_utils
from concourse.tile import TileContext

F32 = mybir.dt.float32
F32R = mybir.dt.float32r
AF = mybir.ActivationFunctionType

B, T, D, H, HD = 32, 512, 1024, 16, 64
NCORES = 8
BN = B // NCORES          # batches per core = 4
TOK = BN * T              # tokens per core = 2048
NKT = D // 128            # 8 contraction tiles
NMC = TOK // 512          # 4 token chunks (phase A)
MASK_NEG = -60000.0       # exp(-60000 + |logit|) == 0 in fp32

_CACHE = {}


def _build(with_qk_bias: bool):
    nc = bacc.Bacc("TRN2", target_bir_lowering=False, debug=False,
                   num_devices=NCORES)

    xT = nc.dram_tensor("xT", [D, TOK], F32, kind="ExternalInput")
    wi = nc.dram_tensor("wi", [D, D], F32, kind="ExternalInput")
    wq = nc.dram_tensor("wq", [D, D], F32, kind="ExternalInput")
    wk = nc.dram_tensor("wk", [D, D], F32, kind="ExternalInput")
    wv = nc.dram_tensor("wv", [D, D], F32, kind="ExternalInput")
    wo = nc.dram_tensor("wo", [D, D], F32, kind="ExternalInput")
    tri = nc.dram_tensor("tri", [128, 128], F32, kind="ExternalInput")
    onesc = nc.dram_tensor("onesc", [128, H], F32, kind="ExternalInput")
    if with_qk_bias:
        bq2 = nc.dram_tensor("bq2", [128, NKT], F32, kind="ExternalInput")
        bk2 = nc.dram_tensor("bk2", [128, NKT], F32, kind="ExternalInput")
    out = nc.dram_tensor("out", [TOK, D], F32, kind="ExternalOutput")

    # DRAM scratch spills (feature-major q/k, token-major v)
    qT_d = nc.dram_tensor("qT_d", [D, TOK], F32, kind="Internal")
    kT_d = nc.dram_tensor("kT_d", [D, TOK], F32, kind="Internal")
    v_d = nc.dram_tensor("v_d", [TOK, D], F32, kind="Internal")

    with TileContext(nc) as tc:
        with tc.tile_pool(name="const", bufs=1) as cpool:
            tri_sb = cpool.tile([128, 128], F32, tag="tri")
            nc.sync.dma_start(tri_sb[:], tri[:])
            if with_qk_bias:
                bq_sb = cpool.tile([128, NKT], F32, tag="bq")
                bk_sb = cpool.tile([128, NKT], F32, tag="bk")
                nc.sync.dma_start(bq_sb[:], bq2[:])
                nc.sync.dma_start(bk_sb[:], bk2[:])

            qkpool_cm = tc.tile_pool(name="qk", bufs=1)
            qkpool = qkpool_cm.__enter__()

            # ---------------- Phase A: in_proj + QKV, spill to DRAM ----
            with tc.tile_pool(name="wA", bufs=1) as wpool, \
                 tc.tile_pool(name="actA", bufs=1) as apool, \
                 tc.tile_pool(name="psA", bufs=1, space="PSUM") as pspool:
                wi_sb = [wpool.tile([128, D], F32R, tag=f"wi{k}", name=f"wi{k}") for k in range(NKT)]
                wq_sb = [wpool.tile([128, D], F32R, tag=f"wq{k}", name=f"wq{k}") for k in range(NKT)]
                wk_sb = [wpool.tile([128, D], F32R, tag=f"wk{k}", name=f"wk{k}") for k in range(NKT)]
                wv_sb = [wpool.tile([128, D], F32R, tag=f"wv{k}", name=f"wv{k}") for k in range(NKT)]
                # DMA order matters for the startup critical path: the first
                # chunk only needs Wi + x, so issue those first and let the
                # 12 MiB of Wq/Wk/Wv stream in under chunk-0's in_proj.
                xc0 = [apool.tile([128, 512], F32R, tag=f"xc{k}", bufs=1, name=f"xc{k}")
                       for k in range(NKT)]
                for k in range(NKT):
                    nc.sync.dma_start(wi_sb[k][:], wi[128 * k:128 * (k + 1), :].bitcast(F32R))
                    nc.sync.dma_start(
                        xc0[k][:], xT[128 * k:128 * (k + 1), 0:512].bitcast(F32R))

                for mc in range(NMC):
                    c0 = 512 * mc
                    if mc == 0:
                        xc = xc0
                    else:
                        xc = [apool.tile([128, 512], F32R, tag=f"xc{k}", bufs=1, name=f"xc{k}")
                              for k in range(NKT)]
                        for k in range(NKT):
                            nc.sync.dma_start(
                                xc[k][:], xT[128 * k:128 * (k + 1), c0:c0 + 512].bitcast(F32R))
                    if mc == 0:
                        for k in range(NKT):
                            nc.sync.dma_start(wq_sb[k][:], wq[128 * k:128 * (k + 1), :].bitcast(F32R))
                            nc.sync.dma_start(wk_sb[k][:], wk[128 * k:128 * (k + 1), :].bitcast(F32R))
                            nc.sync.dma_start(wv_sb[k][:], wv[128 * k:128 * (k + 1), :].bitcast(F32R))

                    # h0^T chunk [1024, 512] feature-major
                    h0 = [apool.tile([128, 512], F32R, tag=f"h0{n}", bufs=1, name=f"h0{n}")
                          for n in range(NKT)]
                    for n in range(NKT):
                        ph = pspool.tile([128, 512], F32, tag="ps", bufs=7)
                        for k in range(NKT):
                            nc.tensor.matmul(
                                ph[:], wi_sb[k][:, 128 * n:128 * (n + 1)], xc[k][:],
                                start=(k == 0), stop=(k == NKT - 1))
                        nc.vector.tensor_copy(h0[n][:], ph[:])

                    # q^T / k^T chunks (feature-major), spilled
                    for w_sb, b_ap, dst in (
                        (wq_sb, "bq", qT_d), (wk_sb, "bk", kT_d)):
                        for n in range(NKT):
                            pq = pspool.tile([128, 512], F32, tag="ps", bufs=7)
                            for k in range(NKT):
                                nc.tensor.matmul(
                                    pq[:], w_sb[k][:, 128 * n:128 * (n + 1)], h0[k][:],
                                    start=(k == 0), stop=(k == NKT - 1))
                            dst_ap = dst[128 * n:128 * (n + 1), c0:c0 + 512]
                            qs = apool.tile([128, 512], F32, tag="spill", bufs=4,
                                            name=f"qs{n}")
                            if with_qk_bias:
                                bias = (bq_sb if b_ap == "bq" else bk_sb)[:, n:n + 1]
                                nc.vector.tensor_scalar_add(qs[:], pq[:], bias)
                            else:
                                nc.vector.tensor_copy(qs[:], pq[:])
                            nc.sync.dma_start(dst_ap, qs[:])

                    # v chunk (token-major), spilled
                    for tt in range(4):
                        for nn in range(2):
                            pv = pspool.tile([128, 512], F32, tag="ps", bufs=7)
                            for k in range(NKT):
                                nc.tensor.matmul(
                                    pv[:], h0[k][:, 128 * tt:128 * (tt + 1)],
                                    wv_sb[k][:, 512 * nn:512 * (nn + 1)],
                                    start=(k == 0), stop=(k == NKT - 1))
                            vs = apool.tile([128, 512], F32, tag="spill", bufs=4,
                                            name=f"vs{tt}{nn}")
                            nc.vector.tensor_copy(vs[:], pv[:])
                            nc.sync.dma_start(
                                v_d[c0 + 128 * tt:c0 + 128 * (tt + 1),
                                    512 * nn:512 * (nn + 1)], vs[:])

            # ---------------- Phase B: attention + out_proj per batch --
            with tc.tile_pool(name="wB", bufs=1) as wpool, \
                 tc.tile_pool(name="actB", bufs=1) as apool, \
                 tc.tile_pool(name="psB", bufs=1, space="PSUM") as psB:
                psS = psO = psF = psB
                wo_sb = [wpool.tile([128, D], F32R, tag=f"wo{k}", name=f"wo{k}") for k in range(NKT)]
                wo_loaded = [False]

                def load_wo():
                    for k in range(NKT):
                        nc.sync.dma_start(wo_sb[k][:], wo[128 * k:128 * (k + 1), :].bitcast(F32R))
                    wo_loaded[0] = True

                def attention(b):
                    """Emit qkv loads + 16 heads of attention for batch b.
                    Returns the normalized oT tiles."""
                    r0 = 512 * b
                    qt = [qkpool.tile([128, 512], F32R, tag=f"qt{e}", bufs=1, name=f"qt{e}")
                          for e in range(NKT)]
                    kt = [qkpool.tile([128, 512], F32R, tag=f"kt{e}", bufs=1, name=f"kt{e}")
                          for e in range(NKT)]
                    for e in range(NKT):
                        nc.sync.dma_start(
                            qt[e][:], qT_d[128 * e:128 * (e + 1), r0:r0 + 512].bitcast(F32R))
                        nc.sync.dma_start(
                            kt[e][:], kT_d[128 * e:128 * (e + 1), r0:r0 + 512].bitcast(F32R))
                    # v_plus tiles: [128, H, 65] = per-head 64 v cols + ones col
                    vp = [apool.tile([128, H * 65], F32R, tag=f"vp{i}", bufs=2, name=f"vp{i}")
                          for i in range(4)]
                    for i in range(4):
                        v3 = vp[i][:].rearrange("p (h e) -> p h e", e=65)
                        nc.sync.dma_start(
                            v3[:, :, 0:64],
                            v_d[r0 + 128 * i:r0 + 128 * (i + 1), :]
                            .bitcast(F32R).rearrange("p (h e) -> p h e", e=64))
                        nc.sync.dma_start(v3[:, :, 64], onesc[:, :].bitcast(F32R))
                    if not wo_loaded[0]:
                        load_wo()

                    oT = [apool.tile([128, 512], F32R, tag=f"oT{e}", bufs=2, name=f"oT{e}")
                          for e in range(NKT)]
                    # process heads in base-partition pairs (rows 0-63 /
                    # 64-127 of the same e-tile -> distinct PE row groups)
                    for m in range(H // 2):
                        et = m
                        pos = {}
                        pts = {}
                        for i in range(4):
                            w0 = 128 * i  # valid t-cols are [w0, 512)
                            for j in (2 * m, 2 * m + 1):
                                off = 64 * (j % 2)
                                ps = psS.tile([128, 512], F32, tag="ps", bufs=5,
                                              name=f"ps{i}{j % 2}")
                                nc.tensor.matmul(
                                    ps[:, w0:512],
                                    qt[et][off:off + 64, w0:w0 + 128],
                                    kt[et][off:off + 64, w0:512],
                                    start=True, stop=True)
                                # causal mask on the diagonal block (s > t)
                                nc.vector.tensor_add(
                                    ps[:, w0:w0 + 128], ps[:, w0:w0 + 128], tri_sb[:])
                                pt = apool.tile([128, 512], F32R, tag="pt", bufs=10,
                                                name=f"pt{i}{j % 2}")
                                nc.scalar.activation(pt[:, w0:512], ps[:, w0:512], AF.Exp)
                                pts[(j, i)] = pt
                        for j in (2 * m, 2 * m + 1):
                            po = psO.tile([65, 512], F32, tag="po", bufs=2, name=f"po{j % 2}")
                            pos[j] = po
                            for i in range(4):
                                w0 = 128 * i
                                nc.tensor.matmul(
                                    po[0:65, w0:512],
                                    vp[i][:, 65 * j:65 * (j + 1)],
                                    pts[(j, i)][:, w0:512],
                                    start=(i == 0), stop=(i == 3), skip_group_check=True)
                        # normalize the pair: O^T[e,t] / denom[t].
                        # partition_broadcast reads absolute partition 0, so
                        # each head's reciprocal lives in its own tile.
                        for j in (2 * m, 2 * m + 1):
                            off = 64 * (j % 2)
                            rs = apool.tile([1, 512], F32R, tag="rs", bufs=4, name="rs")
                            with nc.allow_low_precision(reason="f32r softmax recip"):
                                nc.vector.reciprocal(rs[:], pos[j][64:65, :])
                            rb = apool.tile([64, 512], F32R, tag="rb", bufs=4, name="rb")
                            nc.gpsimd.partition_broadcast(rb[:], rs[:])
                            nc.vector.tensor_mul(oT[et][off:off + 64, :],
                                                 pos[j][0:64, :], rb[:])
                    return oT

                def out_proj(b, oT):
                    """out[t, n] = oT-chain @ Wo for batch b (token-major)."""
                    r0 = 512 * b
                    for tt in range(4):
                        for nn in range(2):
                            pf = psF.tile([128, 512], F32, tag="pf", bufs=1)
                            for k in range(NKT):
                                nc.tensor.matmul(
                                    pf[:], oT[k][:, 128 * tt:128 * (tt + 1)],
                                    wo_sb[k][:, 512 * nn:512 * (nn + 1)],
                                    start=(k == 0), stop=(k == NKT - 1))
                            os_ = apool.tile([128, 512], F32, tag="os", bufs=3,
                                             name=f"os{tt}{nn}")
                            nc.vector.tensor_copy(os_[:], pf[:])
                            nc.sync.dma_start(
                                out[r0 + 128 * tt:r0 + 128 * (tt + 1),
                                    512 * nn:512 * (nn + 1)], os_[:])

                # interleave: attention(b+1) is emitted before out_proj(b) so
                # PE always has ready matmuls while batch b's normalization
                # tail resolves.
                prev = (0, attention(0))
                for b in range(1, BN):
                    cur = (b, attention(b))
                    out_proj(*prev)
                    prev = cur
                out_proj(*prev)
            qkpool_cm.__exit__(None, None, None)

    nc.compile()
    return nc


def _ensure_built(with_qk_bias: bool):
    if with_qk_bias not in _CACHE:
        _CACHE[with_qk_bias] = _build(with_qk_bias)
    return _CACHE[with_qk_bias]


def _prepare(x, Wi, bi, Wk, bk, Wq, bq, Wv, bv, Wo, bo):
    """Host-side prep: returns (in_maps, out_const, with_qk_bias)."""
    x, Wi, bi = np.asarray(x, np.float32), np.asarray(Wi, np.float32), np.asarray(bi, np.float32)
    Wk, bk = np.asarray(Wk, np.float32), np.asarray(bk, np.float32)
    Wq, bq = np.asarray(Wq, np.float32), np.asarray(bq, np.float32)
    Wv, bv = np.asarray(Wv, np.float32), np.asarray(bv, np.float32)
    Wo, bo = np.asarray(Wo, np.float32), np.asarray(bo, np.float32)

    # flatten head-stacked weights: col f = h*HD + e
    wq_f = np.ascontiguousarray(Wq.transpose(1, 0, 2).reshape(D, D))
    wk_f = np.ascontiguousarray(Wk.transpose(1, 0, 2).reshape(D, D))
    wv_f = np.ascontiguousarray(Wv.transpose(1, 0, 2).reshape(D, D))
    # fold bi through the qkv projections; fold bv through out_proj
    bq_fold = (bi @ wq_f + bq.reshape(-1)).astype(np.float32)
    bk_fold = (bi @ wk_f + bk.reshape(-1)).astype(np.float32)
    bv_fold = (bi @ wv_f + bv.reshape(-1)).astype(np.float32)
    out_const = (bv_fold @ Wo + bo).astype(np.float32)  # added host-side

    with_qk_bias = bool(np.any(bq_fold) or np.any(bk_fold))

    tri_add = ((np.triu(np.ones((128, 128))) - 1.0) * -MASK_NEG).astype(np.float32)
    onesc = np.ones((128, H), np.float32)

    shared = {"wi": Wi, "wq": wq_f, "wk": wk_f, "wv": wv_f, "wo": Wo,
              "tri": tri_add, "onesc": onesc}
    if with_qk_bias:
        shared["bq2"] = np.ascontiguousarray(bq_fold.reshape(NKT, 128).T)
        shared["bk2"] = np.ascontiguousarray(bk_fold.reshape(NKT, 128).T)

    in_maps = []
    for c in range(NCORES):
        xs = x[BN * c:BN * (c + 1)].reshape(TOK, D)
        m = dict(shared)
        m["xT"] = np.ascontiguousarray(xs.T)
        in_maps.append(m)
    return in_maps, out_const, with_qk_bias


def kernel(x, Wi, bi, Wk, bk, Wq, bq, Wv, bv, Wo, bo):
    in_maps, out_const, with_qk_bias = _prepare(
        x, Wi, bi, Wk, bk, Wq, bq, Wv, bv, Wo, bo)
    nc = _ensure_built(with_qk_bias)
    res = bass_utils.run_bass_kernel_spmd(nc, in_maps, core_ids=list(range(NCORES)))
    outs = [res.results[c]["out"] for c in range(NCORES)]
    full = np.concatenate(outs, axis=0).reshape(B, T, D)
    full += out_const[None, None, :]
    return full
